# revision 7
# baseline (speedup 1.0000x reference)
"""BCQConv1D TRN2 kernel: out[b,s,o] = x[b,s,:] @ W[o,:]^T + bias[o],
W[o, g*A+a] = sum_qb alpha[o,g,qb] * binary[o,g,a,qb].

Column-parallel sharding: out_features split 8 ways across 8 NeuronCores;
x replicated (each core computes out[:, o_shard]).

Per core: reconstruct the W^T shard on device (DVE fused mul-add over the
3 bit planes + PE transpose), keep it resident in SBUF as float32r, then
run N=512 float32r matmuls (full bf16 rate on the PE) accumulating over
K=4096 in PSUM. The first NSPLIT bs-chunks split the K loop in half with
an SBUF spill so the matmul stream overlaps the weight-reconstruction DMA
instead of waiting for the last W^T tile. Bias is broadcast once via a
K=1 ones-matmul and folded into the spill / output add.

PSUM budget: 6 banks for matmul chains (4 per chunk + 2 lookahead),
2 banks for recon transposes + bias bootstrap.

Host side only slices/relayouts inputs (x is passed transposed/tiled
[128, KT, BS] so every DMA line is 2KB contiguous; xt DMAs fetch 4
k-tiles at once = 1MB per transfer).
"""

import numpy as np

import concourse.bass as bass
import concourse.tile as tile
from concourse import bacc, mybir
from concourse.bass_utils import run_bass_kernel_spmd
from concourse.masks import make_identity

# Problem shape (hardcoded per contest contract)
B, S, I, O = 4, 2048, 4096, 4096
G, A, QB = 32, 128, 3
BS = B * S  # 8192
P = 128
KT = I // P  # 32 k-tiles (== groups: i = g*A + a, A == P)

# Sharding / tiling
N_CORES = 8
O_WAYS = 8
BS_WAYS = 1
O_SH = O // O_WAYS  # 512
BS_SH = BS // BS_WAYS  # 8192
NFREE = 512  # matmul moving free dim (one PSUM bank of fp32)
NB = O_SH // NFREE  # 1 o-block per core
BCHUNK = 512  # bs columns per chunk
NSUB = BCHUNK // P  # 4 matmul chains per chunk
GMERGE = 2  # binary groups per DMA (392KB transfers)
KMERGE = 1  # k-tiles per xt DMA
KSPLIT = KT // 2  # split point of the K loop for leading chunks
NSPLIT = 5  # leading chunks that use the split-K spill (recon overlap)

F32 = mybir.dt.float32
F32R = mybir.dt.float32r


def build_nc():
    nc = bacc.Bacc(target_bir_lowering=False)
    xt_d = nc.declare_dram_parameter("xt", [P, KT, BS_SH], F32R, isOutput=False)
    alpha_d = nc.declare_dram_parameter("alpha", [O_SH, G, QB], F32, isOutput=False)
    binary_d = nc.declare_dram_parameter("binary", [O_SH, G, A, QB], F32, isOutput=False)
    bias_d = nc.declare_dram_parameter("bias", [O_SH], F32, isOutput=False)
    out_d = nc.declare_dram_parameter("out", [BS_SH, O_SH], F32, isOutput=True)

    OT = O_SH // P  # o-tiles for recon
    add = mybir.AluOpType.add
    mult = mybir.AluOpType.mult
    n_chunks = BS_SH // BCHUNK

    with tile.TileContext(nc) as tc:
        with (
            tc.tile_pool(name="const", bufs=1) as cpool,
            tc.tile_pool(name="wt", bufs=1) as wtpool,
            tc.tile_pool(name="acc", bufs=1) as accpool,
            tc.tile_pool(name="rec", bufs=3) as rec,
            tc.tile_pool(name="wog", bufs=4) as wog_pool,
            tc.tile_pool(name="xp", bufs=4) as xp,
            tc.tile_pool(name="op", bufs=4) as op,
            tc.tile_pool(name="pmm", bufs=6, space="PSUM") as pmm,
            tc.tile_pool(name="prec", bufs=2, space="PSUM") as prec,
        ):
            # --- constants ---
            ident = cpool.tile([P, P], F32, name="ident")
            make_identity(nc, ident)
            ones = cpool.tile([1, P], F32, name="ones")
            nc.vector.memset(ones, 1.0)
            bias_row = cpool.tile([1, O_SH], F32, name="bias_row")
            nc.sync.dma_start(out=bias_row, in_=bias_d.ap().unsqueeze(0))
            bias_bc = cpool.tile([P, O_SH], F32, name="bias_bc")
            for j in range(NB):
                pbt = prec.tile([P, NFREE], F32, tag="pr", name=f"psb{j}")
                nc.tensor.matmul(
                    pbt, ones, bias_row[:, j * NFREE : (j + 1) * NFREE],
                    start=True, stop=True,
                )
                nc.vector.tensor_copy(
                    out=bias_bc[:, j * NFREE : (j + 1) * NFREE], in_=pbt
                )

            # --- alpha (per-partition scalars), all o-tiles resident ---
            alpha_sb = []
            for ot in range(OT):
                at = cpool.tile([P, G, QB], F32, name=f"alpha{ot}")
                nc.sync.dma_start(out=at, in_=alpha_d.ap()[ot * P : (ot + 1) * P])
                alpha_sb.append(at)

            # --- W^T shard, resident, one tile per k-tile (== group) ---
            wt_tiles = [
                wtpool.tile([P, O_SH], F32R, tag=f"wt{k}", name=f"wt{k}")
                for k in range(KT)
            ]

            # --- reconstruction: W[o, g*A + a] then PE-transpose to W^T ---
            for gp in range(G // GMERGE):
                for ot in range(OT):
                    bt = rec.tile([P, GMERGE, A, QB], F32, tag="bt")
                    nc.sync.dma_start(
                        out=bt,
                        in_=binary_d.ap()[
                            ot * P : (ot + 1) * P,
                            gp * GMERGE : (gp + 1) * GMERGE,
                        ],
                    )
                    at = alpha_sb[ot]
                    for gg in range(GMERGE):
                        g = gp * GMERGE + gg
                        w_og = wog_pool.tile([P, P], F32, tag="wog")
                        nc.vector.tensor_scalar_mul(
                            w_og, bt[:, gg, :, 0], at[:, g, 0:1]
                        )
                        nc.vector.scalar_tensor_tensor(
                            w_og, bt[:, gg, :, 1], at[:, g, 1:2], w_og, mult, add
                        )
                        nc.vector.scalar_tensor_tensor(
                            w_og, bt[:, gg, :, 2], at[:, g, 2:3], w_og, mult, add
                        )
                        ptt = prec.tile([P, P], F32, tag="pr", name=f"ptr{g}_{ot}")
                        nc.tensor.transpose(ptt, w_og, ident)
                        nc.vector.tensor_copy(
                            out=wt_tiles[g][:, ot * P : (ot + 1) * P], in_=ptt
                        )

            # --- main matmul ---
            def mm_pass(c, k0, k1, phase):
                psums = [
                    [
                        pmm.tile([P, NFREE], F32, tag="ps", name=f"mm{phase}_{c}_{s}_{j}")
                        for j in range(NB)
                    ]
                    for s in range(NSUB)
                ]
                for kk in range(k0, k1, KMERGE):
                    xt_t = xp.tile([P, KMERGE, BCHUNK], F32R, tag="xt")
                    nc.sync.dma_start(
                        out=xt_t,
                        in_=xt_d.ap()[:, kk : kk + KMERGE, c * BCHUNK : (c + 1) * BCHUNK],
                    )
                    for k4 in range(KMERGE):
                        k = kk + k4
                        for s in range(NSUB):
                            for j in range(NB):
                                nc.tensor.matmul(
                                    psums[s][j],
                                    xt_t[:, k4, s * P : (s + 1) * P],
                                    wt_tiles[k][:, j * NFREE : (j + 1) * NFREE],
                                    start=(k == k0),
                                    stop=(k == k1 - 1),
                                )
                return psums

            def emit_out(c, s, j, os_t):
                nc.sync.dma_start(
                    out=out_d.ap()[
                        c * BCHUNK + s * P : c * BCHUNK + (s + 1) * P,
                        j * NFREE : (j + 1) * NFREE,
                    ],
                    in_=os_t,
                )

            # leading chunks: first K half, spilled (overlaps recon DMA)
            accs = {}
            for c in range(NSPLIT):
                psums = mm_pass(c, 0, KSPLIT, 1)
                for s in range(NSUB):
                    for j in range(NB):
                        acc = accpool.tile(
                            [P, NFREE], F32, tag=f"acc{c}_{s}_{j}", name=f"acc{c}_{s}_{j}"
                        )
                        nc.vector.tensor_tensor(
                            out=acc,
                            in0=psums[s][j],
                            in1=bias_bc[:, j * NFREE : (j + 1) * NFREE],
                            op=add,
                        )
                        accs[(c, s, j)] = acc

            # remaining chunks: full 32-k chains
            for c in range(NSPLIT, n_chunks):
                psums = mm_pass(c, 0, KT, 1)
                for s in range(NSUB):
                    for j in range(NB):
                        os_t = op.tile([P, NFREE], F32, tag="os")
                        nc.vector.tensor_tensor(
                            out=os_t,
                            in0=psums[s][j],
                            in1=bias_bc[:, j * NFREE : (j + 1) * NFREE],
                            op=add,
                        )
                        emit_out(c, s, j, os_t)

            # second K half of the leading chunks
            for c in range(NSPLIT):
                psums = mm_pass(c, KSPLIT, KT, 0)
                for s in range(NSUB):
                    for j in range(NB):
                        os_t = op.tile([P, NFREE], F32, tag="os")
                        nc.vector.tensor_tensor(
                            out=os_t,
                            in0=psums[s][j],
                            in1=accs[(c, s, j)],
                            op=add,
                        )
                        emit_out(c, s, j, os_t)

    if not nc.is_finalized():
        nc.finalize()
    return nc


def shard_inputs(x, alpha, bias, binary):
    """Host-side slicing/relayout only. Returns per-core input maps."""
    x2 = np.ascontiguousarray(x).reshape(BS, I)
    # xtp[p, k, s] = x2[s, k*P + p]  -> every DMA line is bs-contiguous
    xtp = np.ascontiguousarray(x2.T.reshape(KT, P, BS).transpose(1, 0, 2))
    alpha = np.ascontiguousarray(alpha)
    binary = np.ascontiguousarray(binary)
    bias = np.ascontiguousarray(bias)

    xparts = [
        xtp if BS_WAYS == 1
        else np.ascontiguousarray(xtp[:, :, bc * BS_SH : (bc + 1) * BS_SH])
        for bc in range(BS_WAYS)
    ]

    in_maps = []
    for c in range(N_CORES):
        oc, bc = divmod(c, BS_WAYS)
        osl = slice(oc * O_SH, (oc + 1) * O_SH)
        in_maps.append(
            {
                "xt": xparts[bc],
                "alpha": alpha[osl],
                "binary": binary[osl],
                "bias": bias[osl],
            }
        )
    return in_maps


def assemble_output(results):
    out = np.empty((BS, O), dtype=np.float32)
    for c in range(N_CORES):
        oc, bc = divmod(c, BS_WAYS)
        out[
            bc * BS_SH : (bc + 1) * BS_SH, oc * O_SH : (oc + 1) * O_SH
        ] = results[c]["out"]
    return out.reshape(B, S, O)


_NC_CACHE = None


def kernel(x, alpha, bias, binary):
    global _NC_CACHE
    if _NC_CACHE is None:
        _NC_CACHE = build_nc()
    nc = _NC_CACHE
    in_maps = shard_inputs(
        np.asarray(x, dtype=np.float32),
        np.asarray(alpha, dtype=np.float32),
        np.asarray(bias, dtype=np.float32),
        np.asarray(binary, dtype=np.float32),
    )
    res = run_bass_kernel_spmd(nc, in_maps, list(range(N_CORES)))
    return assemble_output(res.results)


# revision 8
# speedup vs baseline: 1.6596x; 1.6596x over previous
"""BCQConv1D TRN2 kernel: out[b,s,o] = x[b,s,:] @ W[o,:]^T + bias[o],
W[o, g*A+a] = sum_qb alpha[o,g,qb] * binary[o,g,a,qb].

Column-parallel sharding: out_features split 8 ways across 8 NeuronCores;
x replicated (each core computes out[:, o_shard]).

Per core: reconstruct the W^T shard on device (DVE fused mul-add over the
3 bit planes + PE transpose), keep it resident in SBUF as float32r, then
run N=512 float32r matmuls (full bf16 rate on the PE) accumulating over
K=4096 in PSUM. The first NSPLIT bs-chunks split the K loop in half with
an SBUF spill so the matmul stream overlaps the weight-reconstruction DMA
instead of waiting for the last W^T tile. Bias is broadcast once via a
K=1 ones-matmul and folded into the spill / output add.

PSUM budget: 6 banks for matmul chains (4 per chunk + 2 lookahead),
2 banks for recon transposes + bias bootstrap.

Host side only slices/relayouts inputs (x is passed transposed/tiled
[128, KT, BS] so every DMA line is 2KB contiguous; xt DMAs fetch 4
k-tiles at once = 1MB per transfer).
"""

import numpy as np

import concourse.bass as bass
import concourse.tile as tile
from concourse import bacc, mybir
from concourse.bass_utils import run_bass_kernel_spmd
from concourse.masks import make_identity

# Problem shape (hardcoded per contest contract)
B, S, I, O = 4, 2048, 4096, 4096
G, A, QB = 32, 128, 3
BS = B * S  # 8192
P = 128
KT = I // P  # 32 k-tiles (== groups: i = g*A + a, A == P)

# Sharding / tiling
N_CORES = 8
O_WAYS = 8
BS_WAYS = 1
O_SH = O // O_WAYS  # 512
BS_SH = BS // BS_WAYS  # 8192
NFREE = 512  # matmul moving free dim (one PSUM bank of fp32)
NB = O_SH // NFREE  # 1 o-block per core
BCHUNK = 512  # bs columns per chunk
NSUB = BCHUNK // P  # 4 matmul chains per chunk
GMERGE = 2  # binary groups per DMA (392KB transfers)
KMERGE = 4  # k-tiles per xt DMA (1MB transfers)
KSPLIT = KT // 2  # split point of the K loop for leading chunks
NSPLIT = 5  # leading chunks that use the split-K spill (recon overlap)

F32 = mybir.dt.float32
F32R = mybir.dt.float32r


def build_nc():
    nc = bacc.Bacc(target_bir_lowering=False)
    xt_d = nc.declare_dram_parameter("xt", [P, KT, BS_SH], F32R, isOutput=False)
    alpha_d = nc.declare_dram_parameter("alpha", [O_SH, G, QB], F32, isOutput=False)
    binary_d = nc.declare_dram_parameter("binary", [O_SH, G, A, QB], F32, isOutput=False)
    bias_d = nc.declare_dram_parameter("bias", [O_SH], F32, isOutput=False)
    out_d = nc.declare_dram_parameter("out", [BS_SH, O_SH], F32, isOutput=True)

    OT = O_SH // P  # o-tiles for recon
    add = mybir.AluOpType.add
    mult = mybir.AluOpType.mult
    n_chunks = BS_SH // BCHUNK

    with tile.TileContext(nc) as tc:
        with (
            tc.tile_pool(name="const", bufs=1) as cpool,
            tc.tile_pool(name="wt", bufs=1) as wtpool,
            tc.tile_pool(name="acc", bufs=1) as accpool,
            tc.tile_pool(name="rec", bufs=3) as rec,
            tc.tile_pool(name="wog", bufs=4) as wog_pool,
            tc.tile_pool(name="xp", bufs=4) as xp,
            tc.tile_pool(name="op", bufs=4) as op,
            tc.tile_pool(name="pmm", bufs=6, space="PSUM") as pmm,
            tc.tile_pool(name="prec", bufs=2, space="PSUM") as prec,
        ):
            # --- constants ---
            ident = cpool.tile([P, P], F32, name="ident")
            make_identity(nc, ident)
            ones = cpool.tile([1, P], F32, name="ones")
            nc.vector.memset(ones, 1.0)
            bias_row = cpool.tile([1, O_SH], F32, name="bias_row")
            nc.sync.dma_start(out=bias_row, in_=bias_d.ap().unsqueeze(0))
            bias_bc = cpool.tile([P, O_SH], F32, name="bias_bc")
            for j in range(NB):
                pbt = prec.tile([P, NFREE], F32, tag="pr", name=f"psb{j}")
                nc.tensor.matmul(
                    pbt, ones, bias_row[:, j * NFREE : (j + 1) * NFREE],
                    start=True, stop=True,
                )
                nc.vector.tensor_copy(
                    out=bias_bc[:, j * NFREE : (j + 1) * NFREE], in_=pbt
                )

            # --- alpha (per-partition scalars), all o-tiles resident ---
            alpha_sb = []
            for ot in range(OT):
                at = cpool.tile([P, G, QB], F32, name=f"alpha{ot}")
                nc.sync.dma_start(out=at, in_=alpha_d.ap()[ot * P : (ot + 1) * P])
                alpha_sb.append(at)

            # --- W^T shard, resident, one tile per k-tile (== group) ---
            wt_tiles = [
                wtpool.tile([P, O_SH], F32R, tag=f"wt{k}", name=f"wt{k}")
                for k in range(KT)
            ]

            # --- reconstruction: W[o, g*A + a] then PE-transpose to W^T ---
            for gp in range(G // GMERGE):
                for ot in range(OT):
                    bt = rec.tile([P, GMERGE, A, QB], F32, tag="bt")
                    nc.sync.dma_start(
                        out=bt,
                        in_=binary_d.ap()[
                            ot * P : (ot + 1) * P,
                            gp * GMERGE : (gp + 1) * GMERGE,
                        ],
                    )
                    at = alpha_sb[ot]
                    for gg in range(GMERGE):
                        g = gp * GMERGE + gg
                        w_og = wog_pool.tile([P, P], F32, tag="wog")
                        nc.vector.tensor_scalar_mul(
                            w_og, bt[:, gg, :, 0], at[:, g, 0:1]
                        )
                        nc.vector.scalar_tensor_tensor(
                            w_og, bt[:, gg, :, 1], at[:, g, 1:2], w_og, mult, add
                        )
                        nc.vector.scalar_tensor_tensor(
                            w_og, bt[:, gg, :, 2], at[:, g, 2:3], w_og, mult, add
                        )
                        ptt = prec.tile([P, P], F32, tag="pr", name=f"ptr{g}_{ot}")
                        nc.tensor.transpose(ptt, w_og, ident)
                        nc.vector.tensor_copy(
                            out=wt_tiles[g][:, ot * P : (ot + 1) * P], in_=ptt
                        )

            # --- main matmul ---
            def mm_pass(c, k0, k1, phase):
                psums = [
                    [
                        pmm.tile([P, NFREE], F32, tag="ps", name=f"mm{phase}_{c}_{s}_{j}")
                        for j in range(NB)
                    ]
                    for s in range(NSUB)
                ]
                for kk in range(k0, k1, KMERGE):
                    xt_t = xp.tile([P, KMERGE, BCHUNK], F32R, tag="xt")
                    nc.sync.dma_start(
                        out=xt_t,
                        in_=xt_d.ap()[:, kk : kk + KMERGE, c * BCHUNK : (c + 1) * BCHUNK],
                    )
                    for k4 in range(KMERGE):
                        k = kk + k4
                        for s in range(NSUB):
                            for j in range(NB):
                                nc.tensor.matmul(
                                    psums[s][j],
                                    xt_t[:, k4, s * P : (s + 1) * P],
                                    wt_tiles[k][:, j * NFREE : (j + 1) * NFREE],
                                    start=(k == k0),
                                    stop=(k == k1 - 1),
                                )
                return psums

            def emit_out(c, s, j, os_t):
                nc.sync.dma_start(
                    out=out_d.ap()[
                        c * BCHUNK + s * P : c * BCHUNK + (s + 1) * P,
                        j * NFREE : (j + 1) * NFREE,
                    ],
                    in_=os_t,
                )

            # leading chunks: first K half, spilled (overlaps recon DMA)
            accs = {}
            for c in range(NSPLIT):
                psums = mm_pass(c, 0, KSPLIT, 1)
                for s in range(NSUB):
                    for j in range(NB):
                        acc = accpool.tile(
                            [P, NFREE], F32, tag=f"acc{c}_{s}_{j}", name=f"acc{c}_{s}_{j}"
                        )
                        nc.vector.tensor_tensor(
                            out=acc,
                            in0=psums[s][j],
                            in1=bias_bc[:, j * NFREE : (j + 1) * NFREE],
                            op=add,
                        )
                        accs[(c, s, j)] = acc

            # remaining chunks: full 32-k chains
            for c in range(NSPLIT, n_chunks):
                psums = mm_pass(c, 0, KT, 1)
                for s in range(NSUB):
                    for j in range(NB):
                        os_t = op.tile([P, NFREE], F32, tag="os")
                        nc.vector.tensor_tensor(
                            out=os_t,
                            in0=psums[s][j],
                            in1=bias_bc[:, j * NFREE : (j + 1) * NFREE],
                            op=add,
                        )
                        emit_out(c, s, j, os_t)

            # second K half of the leading chunks
            for c in range(NSPLIT):
                psums = mm_pass(c, KSPLIT, KT, 0)
                for s in range(NSUB):
                    for j in range(NB):
                        os_t = op.tile([P, NFREE], F32, tag="os")
                        nc.vector.tensor_tensor(
                            out=os_t,
                            in0=psums[s][j],
                            in1=accs[(c, s, j)],
                            op=add,
                        )
                        emit_out(c, s, j, os_t)

    if not nc.is_finalized():
        nc.finalize()
    return nc


def shard_inputs(x, alpha, bias, binary):
    """Host-side slicing/relayout only. Returns per-core input maps."""
    x2 = np.ascontiguousarray(x).reshape(BS, I)
    # xtp[p, k, s] = x2[s, k*P + p]  -> every DMA line is bs-contiguous
    xtp = np.ascontiguousarray(x2.T.reshape(KT, P, BS).transpose(1, 0, 2))
    alpha = np.ascontiguousarray(alpha)
    binary = np.ascontiguousarray(binary)
    bias = np.ascontiguousarray(bias)

    xparts = [
        xtp if BS_WAYS == 1
        else np.ascontiguousarray(xtp[:, :, bc * BS_SH : (bc + 1) * BS_SH])
        for bc in range(BS_WAYS)
    ]

    in_maps = []
    for c in range(N_CORES):
        oc, bc = divmod(c, BS_WAYS)
        osl = slice(oc * O_SH, (oc + 1) * O_SH)
        in_maps.append(
            {
                "xt": xparts[bc],
                "alpha": alpha[osl],
                "binary": binary[osl],
                "bias": bias[osl],
            }
        )
    return in_maps


def assemble_output(results):
    out = np.empty((BS, O), dtype=np.float32)
    for c in range(N_CORES):
        oc, bc = divmod(c, BS_WAYS)
        out[
            bc * BS_SH : (bc + 1) * BS_SH, oc * O_SH : (oc + 1) * O_SH
        ] = results[c]["out"]
    return out.reshape(B, S, O)


_NC_CACHE = None


def kernel(x, alpha, bias, binary):
    global _NC_CACHE
    if _NC_CACHE is None:
        _NC_CACHE = build_nc()
    nc = _NC_CACHE
    in_maps = shard_inputs(
        np.asarray(x, dtype=np.float32),
        np.asarray(alpha, dtype=np.float32),
        np.asarray(bias, dtype=np.float32),
        np.asarray(binary, dtype=np.float32),
    )
    res = run_bass_kernel_spmd(nc, in_maps, list(range(N_CORES)))
    return assemble_output(res.results)
